# revision 7
# baseline (speedup 1.0000x reference)
"""GAT layer (B=8, N=2048, IN=128, OUT=64) on 8 trn2 NeuronCores.

Data-parallel: core b handles batch b. Per core:
  h  = x @ W                      [N, OUT]
  e  = lrelu(si + sj), masked     [N, N]   (si = h@a1, sj = h@a2)
  att = softmax_row(e)            [N, N]   (exp w/o max-sub; values bounded)
  hp = att @ h                    [N, OUT]

Engine split: PE does e (rank-2 matmul), transposes, and att@h.
ScalarE does Lrelu + Exp.  VectorE does mask-mult + rowsum (one
tensor_tensor_reduce), reciprocal, and normalize.
"""

import numpy as np
from contextlib import ExitStack

import concourse.bass as bass
import concourse.tile as tile
from concourse import bacc, mybir, masks
from concourse.bass_utils import run_bass_kernel_spmd

F32 = mybir.dt.float32
I32 = mybir.dt.int32

B, N, IN, OUT = 8, 2048, 128, 64
P = 128            # partition tile
NT = N // P        # 16 row tiles
FCH = 512          # psum free chunk
NCH = N // FCH     # 4
ALPHA = 0.2

_CACHE = {}


def _build(adj_bufs=3, att_bufs=3, work_bufs=2, use_lrelu=True):
    nc = bacc.Bacc(
        "TRN2",
        target_bir_lowering=False,
        debug=False,
        enable_asserts=False,
        num_devices=B,
    )
    x_d = nc.dram_tensor("x", [N, IN], F32, kind="ExternalInput").ap()
    adj_d = nc.dram_tensor("adj", [N, N], I32, kind="ExternalInput").ap()
    w_d = nc.dram_tensor("w", [IN, OUT], F32, kind="ExternalInput").ap()
    a_d = nc.dram_tensor("a", [2 * OUT, 1], F32, kind="ExternalInput").ap()
    hp_d = nc.dram_tensor("h_prime", [N, OUT], F32, kind="ExternalOutput").ap()
    att_d = nc.dram_tensor("att", [N, N], F32, kind="ExternalOutput").ap()

    with tile.TileContext(nc) as tc:
        with ExitStack() as ctx:
            _emit(ctx, tc, x_d, adj_d, w_d, a_d, hp_d, att_d,
                  adj_bufs, att_bufs, work_bufs, use_lrelu)
    nc.compile()
    return nc


def _emit(ctx, tc, x_d, adj_d, w_d, a_d, hp_d, att_d,
          adj_bufs, att_bufs, work_bufs, use_lrelu=True):
    nc = tc.nc
    MULT = mybir.AluOpType.mult
    ADD = mybir.AluOpType.add

    const = ctx.enter_context(tc.tile_pool(name="const", bufs=1))
    psA = ctx.enter_context(tc.tile_pool(name="psA", bufs=2, space="PSUM"))
    psB = ctx.enter_context(tc.tile_pool(name="psB", bufs=2, space="PSUM"))
    psC = ctx.enter_context(tc.tile_pool(name="psC", bufs=2, space="PSUM"))
    adj_pool = ctx.enter_context(tc.tile_pool(name="adjp", bufs=adj_bufs))
    work = ctx.enter_context(tc.tile_pool(name="work", bufs=work_bufs))
    att_pool = ctx.enter_context(tc.tile_pool(name="attp", bufs=att_bufs))
    stat = ctx.enter_context(tc.tile_pool(name="stat", bufs=4))
    tsb = ctx.enter_context(tc.tile_pool(name="tsb", bufs=4))
    outp = ctx.enter_context(tc.tile_pool(name="outp", bufs=2))

    ident = const.tile([P, P], F32)
    masks.make_identity(nc, ident[:])

    w_sb = const.tile([IN, OUT], F32)
    nc.sync.dma_start(w_sb[:], w_d)
    a_sb = const.tile([2 * OUT, 1], F32)
    nc.sync.dma_start(a_sb[:], a_d)
    # selector matrices over extended contraction [hT; ones_row]:
    #   S_all = M_S^T @ hT_ext -> [si; ones],  R_all = M_R^T @ hT_ext -> [ones; sj]
    M_S = const.tile([OUT + 1, 2], F32)
    M_R = const.tile([OUT + 1, 2], F32)
    nc.vector.memset(M_S[:], 0.0)
    nc.vector.memset(M_R[:], 0.0)
    nc.vector.tensor_copy(M_S[0:OUT, 0:1], a_sb[0:OUT, :])
    nc.vector.memset(M_S[OUT:OUT + 1, 1:2], 1.0)
    nc.vector.memset(M_R[OUT:OUT + 1, 0:1], 1.0)
    nc.vector.tensor_copy(M_R[0:OUT, 1:2], a_sb[OUT:2 * OUT, :])

    # x tiles: x_sb[p, t, k] = x[t*128+p, k]
    x_sb = const.tile([P, NT * IN], F32)
    nc.sync.dma_start(
        x_sb[:].rearrange("p (t k) -> p t k", t=NT),
        x_d.rearrange("(t p) k -> p t k", p=P),
    )

    # transpose x tiles -> xT_sb[k, t*128+n]
    xT_sb = const.tile([P, N], F32)
    for t in range(NT):
        xt_ps = psB.tile([P, P], F32, tag="psB")
        nc.tensor.matmul(xt_ps[:], x_sb[:, bass.ts(t, IN)], ident[:],
                         is_transpose=True)
        nc.scalar.copy(xT_sb[:, bass.ts(t, P)], xt_ps[:])

    # h tiles (natural): h_sb[n_p, t*64+d]
    h_sb = const.tile([P, NT * OUT], F32)
    for t in range(NT):
        h_ps = psC.tile([P, OUT], F32, tag="psC")
        nc.tensor.matmul(h_ps[:], xT_sb[:, bass.ts(t, P)], w_sb[:])
        nc.scalar.copy(h_sb[:, bass.ts(t, OUT)], h_ps[:])

    # hT_ext [65, N]: rows 0-63 = hT = W^T @ xT, row 64 = ones
    hT_sb = const.tile([OUT + 1, N], F32)
    nc.vector.memset(hT_sb[OUT:OUT + 1, :], 1.0)
    for c in range(NCH):
        ht_ps = psA.tile([OUT, FCH], F32, tag="psA")
        nc.tensor.matmul(ht_ps[:], w_sb[:], xT_sb[:, bass.ts(c, FCH)])
        nc.scalar.copy(hT_sb[0:OUT, bass.ts(c, FCH)], ht_ps[:])

    S_all = const.tile([2, N], F32)   # row0 = si, row1 = ones
    R_all = const.tile([2, N], F32)   # row0 = ones, row1 = sj
    for c in range(NCH):
        s_ps = psA.tile([2, FCH], F32, tag="psA")
        nc.tensor.matmul(s_ps[:], M_S[:], hT_sb[:, bass.ts(c, FCH)])
        nc.scalar.copy(S_all[:, bass.ts(c, FCH)], s_ps[:])
        r_ps = psA.tile([2, FCH], F32, tag="psA")
        nc.tensor.matmul(r_ps[:], M_R[:], hT_sb[:, bass.ts(c, FCH)])
        nc.scalar.copy(R_all[:, bass.ts(c, FCH)], r_ps[:])

    BIG = 1.0e9
    AF = mybir.ActivationFunctionType
    for i in range(NT):
        adj_t = adj_pool.tile([P, N], I32, tag="adj")
        nc.sync.dma_start(adj_t[:], adj_d[i * P:(i + 1) * P, :])
        # additive mask: 0 where adj=1, -BIG where adj=0 (int32 in, f32 out)
        mb = adj_pool.tile([P, N], F32, tag="mb")
        nc.gpsimd.tensor_scalar(mb[:], adj_t[:], BIG, -BIG, MULT, ADD)

        # e = si + sj + mask (PE accumulates the mask via identity matmul)
        t1 = work.tile([P, N], F32, tag="t1")
        for c in range(NCH):
            e_ps = psA.tile([P, FCH], F32, tag="psA")
            nc.tensor.matmul(e_ps[:], S_all[:, bass.ts(i, P)],
                             R_all[:, bass.ts(c, FCH)], start=True, stop=False)
            nc.tensor.matmul(e_ps[:], ident[:], mb[:, bass.ts(c, FCH)],
                             start=False, stop=True)
            fn = AF.Prelu if use_lrelu else AF.Copy
            nc.scalar.activation(t1[:, bass.ts(c, FCH)], e_ps[:], fn,
                                 alpha=ALPHA if use_lrelu else 0.0)
        # exp with free row-sum (masked entries exp(-2e8) -> 0 exactly)
        u = work.tile([P, N], F32, tag="u")
        ssum = stat.tile([P, 1], F32, tag="ssum")
        nc.scalar.activation(u[:], t1[:], AF.Exp, accum_out=ssum[:])
        rec = stat.tile([P, 1], F32, tag="rec")
        nc.vector.reciprocal(rec[:], ssum[:])
        att_t = att_pool.tile([P, N], F32, tag="att")
        nc.vector.tensor_scalar(att_t[:], u[:], rec[:], None, op0=MULT)
        nc.sync.dma_start(att_d[i * P:(i + 1) * P, :], att_t[:])

        # hp tile = att_t @ h  (transpose att blocks, accumulate)
        hp_ps = psC.tile([P, OUT], F32, tag="psC")
        for j in range(NT):
            at_ps = psB.tile([P, P], F32, tag="psB")
            nc.tensor.matmul(at_ps[:], att_t[:, bass.ts(j, P)], ident[:],
                             is_transpose=True)
            at_sb = tsb.tile([P, P], F32, tag="atT")
            nc.vector.tensor_copy(at_sb[:], at_ps[:])
            nc.tensor.matmul(hp_ps[:], at_sb[:], h_sb[:, bass.ts(j, OUT)],
                             start=(j == 0), stop=(j == NT - 1))
        hp_t = outp.tile([P, OUT], F32)
        nc.scalar.copy(hp_t[:], hp_ps[:])
        nc.sync.dma_start(hp_d[i * P:(i + 1) * P, :], hp_t[:])


def _get_nc():
    if "nc" not in _CACHE:
        _CACHE["nc"] = _build()
    return _CACHE["nc"]


def run(inputs, trace=False, **trace_kwargs):
    nc = _get_nc()
    x = np.ascontiguousarray(inputs["x"], dtype=np.float32)
    adj = np.ascontiguousarray(inputs["adj"], dtype=np.int32)
    W = np.ascontiguousarray(inputs["W"], dtype=np.float32)
    a = np.ascontiguousarray(inputs["a"], dtype=np.float32)
    in_maps = [
        {"x": x[b], "adj": adj[b], "w": W, "a": a} for b in range(B)
    ]
    res = run_bass_kernel_spmd(nc, in_maps, list(range(B)), trace=trace,
                               **trace_kwargs)
    hp = np.stack([np.asarray(res.results[b]["h_prime"]) for b in range(B)])
    att = np.stack([np.asarray(res.results[b]["att"]) for b in range(B)])
    return (hp, att), res


def kernel(**inputs):
    (hp, att), _ = run(inputs, trace=False)
    return hp, att
